# revision 32
# baseline (speedup 1.0000x reference)
"""Trainium2 Bass kernel for nn_MultiHeadLinearAttention.

Sharding: batch (4-way) x head-half (2-way) = 8 cores, no cross-core
reductions (the kv / summed_k contractions are over L, which stays local).

Per core (b = core//2, hh = core%2, 8 heads, F=512 local feature cols):
  phase A (per 512-token block, bf16 matmuls, fp32 PSUM accumulation):
    Q^T = fm(Wq_h @ X_q^T + bq)          (feat-major, N=512, bias via ACT)
    K   = fm(X_k Wk_h^T + bk)            (token-major; bias via rank-1 matmul)
    V1  = [(X_v Wv_h^T + bv) * m_kv^2/L | m_kv]
    kv[pair] += K_pair^T @ [V1_even|V1_odd]   (per head pair; accumulated in
        PSUM over all L; diagonal blocks are kv + summed_k mask columns)
  phase B (per 128-token tile): out = (Q^T)^T @ blockdiag(kv) scaled by
    m_q*L / (eps + m_q * (q . summed_k)), summed_k taken from the mask
    column of kv.  The last block's Q projection is deferred into phase B
    so early output tiles overlap its PE work.
fm(y) = elu(y)+1 = min(exp(y),1) + relu(y)   (exact, overflow-safe).
"""

import os
import sys
import types

for _p in ("/opt/trn_rl_repo",):
    if _p not in sys.path:
        sys.path.insert(0, _p)
os.environ.setdefault("MYCRO_LOCAL_CACHE", "1")

import numpy as np
import ml_dtypes

# Optional NTFF profiling support (used by test harness via TRN_KERNEL_TRACE=1).
if "antenv.axon_hooks" not in sys.modules:
    _hook_mod = types.ModuleType("antenv.axon_hooks")
    _hook_state = {"h": None}
    _hook_mod.set_axon_ntff_profile_hook = lambda h: _hook_state.__setitem__("h", h)
    _hook_mod.get_axon_ntff_profile_hook = lambda: _hook_state["h"]
    sys.modules["antenv.axon_hooks"] = _hook_mod
    try:
        from trn_agent_boot.trn_boot import _ntff_profile_via_ctypes
        _hook_state["h"] = _ntff_profile_via_ctypes("/opt/axon/libaxon_pjrt.so")
    except Exception:
        pass

import concourse.bacc as bacc
import concourse.tile as tile
import concourse.mybir as mybir
import concourse.bass_utils as bass_utils
from concourse.bass import ts, ds
from concourse.bass_utils import run_bass_kernel_spmd

bass_utils.upload_artifacts = lambda tmpdir: tmpdir  # no artifact bucket here

F32 = mybir.dt.float32
F32R = mybir.dt.float32r
BF16 = mybir.dt.bfloat16
ACTF = mybir.ActivationFunctionType
ALU = mybir.AluOpType

B, L, C, H, D = 4, 4096, 1024, 16, 64
EPS = 1e-6
NCORES = 8
HPC = H // NCORES * 4          # 8 heads per core
F = HPC * D                    # 512 local feature cols
KC = C // 128                  # 8 contraction chunks
NBLK = L // 512                # 8 token blocks (phase A)
NST = L // 128                 # 32 token subtiles
FT = F // 128                  # 4 feature tiles (head pairs)

_PROG = None


def _emit_kv(nc, b, Ksbs, V1s, kvP0, kvP1):
    # one matmul per head PAIR: lhsT = K columns of both heads (128 wide),
    # rhs = [V1_even | V1_odd] (130 wide).  The diagonal blocks of the
    # (128, 130) output are kv_even / kv_odd (+ mask columns); off-diagonal
    # blocks are cross-head garbage that the kv_sb evacuation skips.
    for s in range(4):
        st = b * 4 + s
        for f in range(FT):
            kvp = kvP0 if f < 2 else kvP1
            nc.tensor.matmul(
                kvp[:, f % 2, :],
                Ksbs[s][:, ts(f, 128)],
                V1s[s][:, 2 * f:2 * f + 2, :].rearrange("p a b -> p (a b)"),
                start=(st == 0 and f % 2 == 0),
                stop=(st == NST - 1 and f % 2 == 1),
                skip_group_check=True)


def _build_program():
    nc = bacc.Bacc("TRN2", target_bir_lowering=False, debug=False,
                   num_devices=NCORES)

    xtq_d = nc.dram_tensor("xtq", [C, L], BF16, kind="ExternalInput")
    xtk_d = nc.dram_tensor("xtk", [C, L], BF16, kind="ExternalInput")
    xtv_d = nc.dram_tensor("xtv", [C, L], BF16, kind="ExternalInput")
    wtq_d = nc.dram_tensor("wtq", [C, F], BF16, kind="ExternalInput")
    wtk_d = nc.dram_tensor("wtk", [C, F], BF16, kind="ExternalInput")
    wtv_d = nc.dram_tensor("wtv", [C, F], BF16, kind="ExternalInput")
    bq_d = nc.dram_tensor("bq", [F], F32, kind="ExternalInput")
    bk_d = nc.dram_tensor("bk", [1, F], BF16, kind="ExternalInput")
    bv_d = nc.dram_tensor("bv", [1, F], BF16, kind="ExternalInput")
    mq_d = nc.dram_tensor("mq", [L], F32, kind="ExternalInput")
    mkv_d = nc.dram_tensor("mkv", [L], F32, kind="ExternalInput")
    out_d = nc.dram_tensor("out", [L, F], F32, kind="ExternalOutput")

    with tile.TileContext(nc) as tc:
        with (
            tc.tile_pool(name="singles", bufs=1) as singles,
            tc.tile_pool(name="xtp", bufs=6) as xtp,
            tc.tile_pool(name="tmp", bufs=4) as tmp,
            tc.tile_pool(name="kvout", bufs=4) as kvout,
        ):
            # constants / weights.  DMA order matters for the pipeline head:
            # the first matmuls need (wtk, xtk block 0) — load those first.
            xtq_ap = xtq_d.ap().rearrange("(o p) l -> p o l", p=128)
            xtk_ap = xtk_d.ap().rearrange("(o p) l -> p o l", p=128)
            xtv_ap = xtv_d.ap().rearrange("(o p) l -> p o l", p=128)

            bk_row = singles.tile([1, F], BF16)
            nc.sync.dma_start(bk_row[:], bk_d.ap())
            wtk_sb = singles.tile([128, KC, F], BF16)
            wtk_ap = wtk_d.ap().rearrange("(o p) f -> p o f", p=128)
            nc.sync.dma_start(wtk_sb[:, 0:2, :], wtk_ap[:, 0:2, :])
            xtk_t0 = xtp.tile([128, KC, 512], BF16, tag="xt")
            nc.sync.dma_start(xtk_t0[:, 0:2, :], xtk_ap[:, 0:2, ds(0, 512)])
            nc.sync.dma_start(wtk_sb[:, 2:4, :], wtk_ap[:, 2:4, :])
            nc.sync.dma_start(xtk_t0[:, 2:4, :], xtk_ap[:, 2:4, ds(0, 512)])
            nc.sync.dma_start(wtk_sb[:, 4:8, :], wtk_ap[:, 4:8, :])
            nc.sync.dma_start(xtk_t0[:, 4:8, :], xtk_ap[:, 4:8, ds(0, 512)])
            bv_row = singles.tile([1, F], BF16)
            nc.sync.dma_start(bv_row[:], bv_d.ap())
            wtv_sb = singles.tile([128, KC, F], BF16)
            nc.sync.dma_start(wtv_sb[:], wtv_d.ap().rearrange("(o p) f -> p o f", p=128))
            xtv_t0 = xtp.tile([128, KC, 512], BF16, tag="xt")
            nc.sync.dma_start(xtv_t0[:], xtv_ap[:, :, ds(0, 512)])
            bq_pp = singles.tile([128, FT], F32)
            nc.sync.dma_start(bq_pp[:], bq_d.ap().rearrange("(f p) -> p f", p=128))
            wtq_sb = singles.tile([128, KC, F], BF16)
            nc.sync.dma_start(wtq_sb[:], wtq_d.ap().rearrange("(o p) f -> p o f", p=128))
            xtq_t0 = xtp.tile([128, KC, 512], BF16, tag="xt")
            nc.sync.dma_start(xtq_t0[:], xtq_ap[:, :, ds(0, 512)])
            ones_row = singles.tile([1, 128], BF16)
            nc.vector.memset(ones_row[:], 1.0)
            mq_sb = singles.tile([128, NST], F32)
            nc.sync.dma_start(mq_sb[:], mq_d.ap().rearrange("(t p) -> p t", p=128))
            mkv_sb = singles.tile([128, NST], F32)
            nc.sync.dma_start(mkv_sb[:], mkv_d.ap().rearrange("(t p) -> p t", p=128))
            # derived mask tiles
            mm2L = singles.tile([128, NST], F32)   # m_kv^2 / L
            nc.vector.scalar_tensor_tensor(mm2L[:], mkv_sb[:], 1.0 / L, mkv_sb[:],
                                           ALU.mult, ALU.mult)
            mqL = singles.tile([128, NST], F32)    # m_q * L
            nc.vector.tensor_scalar(mqL[:], mq_sb[:], float(L), None, ALU.mult)
            mkv8 = singles.tile([128, NST, HPC], F32)  # mask col replicated per head
            for h in range(HPC):
                nc.vector.tensor_copy(out=mkv8[:, :, h:h + 1], in_=mkv_sb[:, :, None])

            QT = singles.tile([128, FT, L], BF16)  # resident Q^T (feat-major)

            with tc.tile_pool(name="psA", bufs=2, space="PSUM") as psA:
                kvP0 = psA.tile([128, 2, 130], F32, name="kvP0", bufs=1)
                kvP1 = psA.tile([128, 2, 130], F32, name="kvP1", bufs=1)
                for b in range(NBLK):
                    with nc.named_scope(f"blk{b}"):
                        if b == 0:
                            xtk_t, xtv_t, xtq_t = xtk_t0, xtv_t0, xtq_t0
                        else:
                            xtk_t = xtp.tile([128, KC, 512], BF16, tag="xt")
                            nc.sync.dma_start(xtk_t[:], xtk_ap[:, :, ds(b * 512, 512)])
                            xtv_t = xtp.tile([128, KC, 512], BF16, tag="xt")
                            nc.sync.dma_start(xtv_t[:], xtv_ap[:, :, ds(b * 512, 512)])
                            xtq_t = xtp.tile([128, KC, 512], BF16, tag="xt")
                            nc.sync.dma_start(xtq_t[:], xtq_ap[:, :, ds(b * 512, 512)])

                        # ---- K projections + feature map, 4 subtiles
                        Ksbs, V1s = [], []
                        for s in range(4):
                            psk = psA.tile([128, 512], F32, tag="psk")
                            for k in range(KC):
                                nc.tensor.matmul(psk[:], xtk_t[:, k, ts(s, 128)],
                                                 wtk_sb[:, k, :],
                                                 start=(k == 0), stop=False)
                            nc.tensor.matmul(psk[:], ones_row[:], bk_row[:],
                                             start=False, stop=True)
                            E2 = tmp.tile([128, 512], F32, tag="E")
                            nc.scalar.activation(E2[:], psk[:], ACTF.Exp)
                            R2 = tmp.tile([128, 512], F32, tag="R")
                            nc.scalar.activation(R2[:], psk[:], ACTF.Relu)
                            Ksb = tmp.tile([128, 512], BF16, tag="Ksb", bufs=6)
                            nc.vector.scalar_tensor_tensor(Ksb[:], E2[:], 1.0,
                                                           R2[:], ALU.min, ALU.add)
                            Ksbs.append(Ksb)

                        # ---- V projections + mask/scale, 4 subtiles
                        for s in range(4):
                            st = b * 4 + s
                            psv = psA.tile([128, 512], F32, tag="psv")
                            for k in range(KC):
                                nc.tensor.matmul(psv[:], xtv_t[:, k, ts(s, 128)],
                                                 wtv_sb[:, k, :],
                                                 start=(k == 0), stop=False)
                            nc.tensor.matmul(psv[:], ones_row[:], bv_row[:],
                                             start=False, stop=True)
                            V1 = tmp.tile([128, HPC, 65], BF16, tag="V1", bufs=6)
                            nc.vector.tensor_scalar(
                                V1[:, :, 0:64],
                                psv.rearrange("p (h d) -> p h d", d=64),
                                mm2L[:, st:st + 1], None, ALU.mult)
                            nc.vector.tensor_copy(out=V1[:, :, 64:65],
                                                  in_=mkv8[:, st, :, None])
                            V1s.append(V1)

                        # ---- kv for blocks 0 and 7 runs before Q (b0: Q data
                        # still in flight; b7: lets early phase-B iterations
                        # overlap the last Q projection)
                        if b in (0, NBLK - 1):
                            _emit_kv(nc, b, Ksbs, V1s, kvP0, kvP1)

                        # ---- Q^T tiles: (128 feat, 512 tok), weights stationary
                        if b == NBLK - 1:
                            last_q = (xtq_t, b)
                            continue
                        for f in range(FT):
                            psq = psA.tile([128, 512], F32, tag="psq")
                            for k in range(KC):
                                nc.tensor.matmul(psq[:], wtq_sb[:, k, ts(f, 128)],
                                                 xtq_t[:, k, :],
                                                 start=(k == 0), stop=(k == KC - 1))
                            E = tmp.tile([128, 512], F32, tag="E")
                            nc.scalar.activation(E[:], psq[:], ACTF.Exp,
                                                 bias=bq_pp[:, f:f + 1])
                            R = tmp.tile([128, 512], F32, tag="R")
                            nc.scalar.activation(R[:], psq[:], ACTF.Relu,
                                                 bias=bq_pp[:, f:f + 1])
                            nc.vector.scalar_tensor_tensor(
                                QT[:, f, ds(b * 512, 512)], E[:], 1.0, R[:],
                                ALU.min, ALU.add)

                        # ---- kv accumulation (blocks 1..6)
                        if b not in (0, NBLK - 1):
                            _emit_kv(nc, b, Ksbs, V1s, kvP0, kvP1)

                # block-diagonal kv for the output einsum:
                # rows 0:64 = even head of pair (cols 0:65), rows 64:128 = odd
                kv_sb = singles.tile([128, FT, 130], BF16)
                nc.vector.memset(kv_sb[:], 0.0)
                nc.vector.tensor_copy(out=kv_sb[0:64, 0:2, 0:65],
                                      in_=kvP0[0:64, :, 0:65])
                nc.vector.tensor_copy(out=kv_sb[64:128, 0:2, 65:130],
                                      in_=kvP0[64:128, :, 65:130])
                nc.vector.tensor_copy(out=kv_sb[0:64, 2:4, 0:65],
                                      in_=kvP1[0:64, :, 0:65])
                nc.vector.tensor_copy(out=kv_sb[64:128, 2:4, 65:130],
                                      in_=kvP1[64:128, :, 65:130])

            with tc.tile_pool(name="psB", bufs=2, space="PSUM") as psB, \
                 nc.named_scope("phaseB"):

                def emit_B2(st0):
                    # two subtiles share one scale-chain (den/rcp/s) pass
                    pos = []
                    qs16 = kvout.tile([128, 2, HPC], F32, tag="qs16", bufs=3,
                                      name="qs16")
                    for j, st in enumerate((st0, st0 + 1)):
                        poA = psB.tile([128, 2, 130], F32, tag="poA", bufs=4)
                        poB = psB.tile([128, 2, 130], F32, tag="poB", bufs=4)
                        for f in range(FT):
                            po = poA if f < 2 else poB
                            nc.tensor.matmul(po[:, f % 2, :],
                                             QT[:, f, ts(st, 128)],
                                             kv_sb[:, f, :],
                                             start=True, stop=True,
                                             skip_group_check=True)
                        nc.vector.tensor_copy(
                            out=qs16[:, j, 0:4].rearrange("p (f c) -> p f c", c=2),
                            in_=poA[:, :, 64:130:65])
                        nc.vector.tensor_copy(
                            out=qs16[:, j, 4:8].rearrange("p (f c) -> p f c", c=2),
                            in_=poB[:, :, 64:130:65])
                        pos.append((poA, poB))
                    den16 = kvout.tile([128, 2, HPC], F32, tag="den16", bufs=3)
                    nc.vector.tensor_tensor(
                        out=den16[:], in0=qs16[:],
                        in1=mq_sb[:, ds(st0, 2), None].to_broadcast((128, 2, HPC)),
                        op=ALU.mult)
                    nc.vector.tensor_scalar(den16[:], den16[:], EPS, None, ALU.add)
                    rcp16 = kvout.tile([128, 2, HPC], F32, tag="rcp16", bufs=3)
                    nc.vector.reciprocal(rcp16[:], den16[:])
                    s16 = kvout.tile([128, 2, HPC], F32, tag="s16", bufs=3)
                    nc.vector.tensor_tensor(
                        out=s16[:], in0=rcp16[:],
                        in1=mqL[:, ds(st0, 2), None].to_broadcast((128, 2, HPC)),
                        op=ALU.mult)
                    for j, st in enumerate((st0, st0 + 1)):
                        emit_B_out(st, pos[j][0], pos[j][1], s16[:, j, :])

                def emit_B_out(st, poA, poB, s8):
                    outsb = kvout.tile([128, HPC, 64], F32, tag="outsb", bufs=4)
                    # poA heads on DVE in one fused 4-D op; poB heads on ScalarE
                    nc.vector.tensor_tensor(
                        out=outsb[:, 0:4, :]
                            .rearrange("p (f par) d -> p f par d", par=2),
                        in0=poA.rearrange("p f (par c) -> p f par c", c=65)
                            [:, :, :, 0:64],
                        in1=s8[:, 0:4]
                            .rearrange("p (f par) -> p f par", par=2)
                            [:, :, :, None].to_broadcast((128, 2, 2, 64)),
                        op=ALU.mult)
                    nc.vector.tensor_tensor(
                        out=outsb[:, 4:6, :]
                            .rearrange("p (f par) d -> p f par d", par=2),
                        in0=poB[:, 0:1, :].rearrange("p f (par c) -> p f par c",
                                                     c=65)[:, :, :, 0:64],
                        in1=s8[:, 4:6]
                            .rearrange("p (f par) -> p f par", par=2)
                            [:, :, :, None].to_broadcast((128, 1, 2, 64)),
                        op=ALU.mult)
                    for h in range(6, 8):
                        par = h % 2
                        cols = slice(0, 64) if par == 0 else slice(65, 129)
                        nc.scalar.activation(outsb[:, h, :], poB[:, 1, cols],
                                             ACTF.Identity, bias=0.0,
                                             scale=s8[:, h:h + 1])
                    nc.sync.dma_start(out_d.ap()[ds(st * 128, 128), :],
                                      outsb.rearrange("p h d -> p (h d)"))

                # first few output tiles run before the deferred last-block Q
                # projection — their epilogues overlap its PE work
                for st0 in range(0, 6, 2):
                    emit_B2(st0)

                xtq_t7, b7 = last_q
                for f in range(FT):
                    psq = psB.tile([128, 512], F32, tag="poA", bufs=4)
                    for k in range(KC):
                        nc.tensor.matmul(psq[:], wtq_sb[:, k, ts(f, 128)],
                                         xtq_t7[:, k, :],
                                         start=(k == 0), stop=(k == KC - 1))
                    E = tmp.tile([128, 512], F32, tag="E")
                    nc.scalar.activation(E[:], psq[:], ACTF.Exp,
                                         bias=bq_pp[:, f:f + 1])
                    R = tmp.tile([128, 512], F32, tag="R")
                    nc.scalar.activation(R[:], psq[:], ACTF.Relu,
                                         bias=bq_pp[:, f:f + 1])
                    nc.vector.scalar_tensor_tensor(
                        QT[:, f, ds(b7 * 512, 512)], E[:], 1.0, R[:],
                        ALU.min, ALU.add)

                for st0 in range(6, NST, 2):
                    emit_B2(st0)

    nc.compile()
    return nc


def _get_program():
    global _PROG
    if _PROG is None:
        _PROG = _build_program()
    return _PROG


def kernel(query, key, value, mask_q, mask_kv, Wq, bq, Wk, bk, Wv, bv):
    query = np.asarray(query, dtype=np.float32)
    key = np.asarray(key, dtype=np.float32)
    value = np.asarray(value, dtype=np.float32)
    mask_q = np.asarray(mask_q, dtype=np.float32)
    mask_kv = np.asarray(mask_kv, dtype=np.float32)
    Wq = np.asarray(Wq, dtype=np.float32)
    bq = np.asarray(bq, dtype=np.float32)
    Wk = np.asarray(Wk, dtype=np.float32)
    bk = np.asarray(bk, dtype=np.float32)
    Wv = np.asarray(Wv, dtype=np.float32)
    bv = np.asarray(bv, dtype=np.float32)

    nc = _get_program()

    bf = ml_dtypes.bfloat16
    xt = {}
    for b in range(B):
        xt[b] = (np.ascontiguousarray(query[b].T).astype(bf),
                 np.ascontiguousarray(key[b].T).astype(bf),
                 np.ascontiguousarray(value[b].T).astype(bf))
    wslices = {}
    for hh in range(2):
        sl = slice(hh * F, (hh + 1) * F)
        wslices[hh] = (
            np.ascontiguousarray(Wq[sl, :].T).astype(bf),
            np.ascontiguousarray(Wk[sl, :].T).astype(bf),
            np.ascontiguousarray(Wv[sl, :].T).astype(bf),
            bq[sl].copy(),
            bk[sl].astype(bf).reshape(1, F),
            bv[sl].astype(bf).reshape(1, F),
        )

    in_maps = []
    for core in range(NCORES):
        b, hh = core // 2, core % 2
        xtq, xtk, xtv = xt[b]
        wtq, wtk, wtv, bq_h, bk_h, bv_h = wslices[hh]
        in_maps.append({
            "xtq": xtq, "xtk": xtk, "xtv": xtv,
            "wtq": wtq, "wtk": wtk, "wtv": wtv,
            "bq": bq_h, "bk": bk_h, "bv": bv_h,
            "mq": mask_q[b], "mkv": mask_kv[b],
        })

    trace = os.environ.get("TRN_KERNEL_TRACE", "0") == "1"
    trace_cores = list(range(NCORES)) if trace else None
    res = run_bass_kernel_spmd(nc, in_maps, list(range(NCORES)),
                               trace=trace, trace_cores=trace_cores)
    if trace:
        kernel.last_exec_time_ns = res.exec_time_ns
        kernel.last_scope_times = res.per_core_scope_times

    out = np.empty((B, L, H, D), dtype=np.float32)
    for core in range(NCORES):
        b, hh = core // 2, core % 2
        out[b, :, hh * HPC:(hh + 1) * HPC, :] = \
            res.results[core]["out"].reshape(L, HPC, D)
    return out


# revision 33
# speedup vs baseline: 1.0092x; 1.0092x over previous
"""Trainium2 Bass kernel for nn_MultiHeadLinearAttention.

Sharding: batch (4-way) x head-half (2-way) = 8 cores, no cross-core
reductions (the kv / summed_k contractions are over L, which stays local).

Per core (b = core//2, hh = core%2, 8 heads, F=512 local feature cols):
  phase A (per 512-token block, bf16 matmuls, fp32 PSUM accumulation):
    Q^T = fm(Wq_h @ X_q^T + bq)          (feat-major, N=512, bias via ACT)
    K   = fm(X_k Wk_h^T + bk)            (token-major; bias via rank-1 matmul)
    V1  = [(X_v Wv_h^T + bv) * m_kv^2/L | m_kv]
    kv[pair] += K_pair^T @ [V1_even|V1_odd]   (per head pair; accumulated in
        PSUM over all L; diagonal blocks are kv + summed_k mask columns)
  phase B (per 128-token tile): out = (Q^T)^T @ blockdiag(kv) scaled by
    m_q*L / (eps + m_q * (q . summed_k)), summed_k taken from the mask
    column of kv.  The last block's Q projection is deferred into phase B
    so early output tiles overlap its PE work.
fm(y) = elu(y)+1 = min(exp(y),1) + relu(y)   (exact, overflow-safe).
"""

import os
import sys
import types

for _p in ("/opt/trn_rl_repo",):
    if _p not in sys.path:
        sys.path.insert(0, _p)
os.environ.setdefault("MYCRO_LOCAL_CACHE", "1")

import numpy as np
import ml_dtypes

# Optional NTFF profiling support (used by test harness via TRN_KERNEL_TRACE=1).
if "antenv.axon_hooks" not in sys.modules:
    _hook_mod = types.ModuleType("antenv.axon_hooks")
    _hook_state = {"h": None}
    _hook_mod.set_axon_ntff_profile_hook = lambda h: _hook_state.__setitem__("h", h)
    _hook_mod.get_axon_ntff_profile_hook = lambda: _hook_state["h"]
    sys.modules["antenv.axon_hooks"] = _hook_mod
    try:
        from trn_agent_boot.trn_boot import _ntff_profile_via_ctypes
        _hook_state["h"] = _ntff_profile_via_ctypes("/opt/axon/libaxon_pjrt.so")
    except Exception:
        pass

import concourse.bacc as bacc
import concourse.tile as tile
import concourse.mybir as mybir
import concourse.bass_utils as bass_utils
from concourse.bass import ts, ds
from concourse.bass_utils import run_bass_kernel_spmd

bass_utils.upload_artifacts = lambda tmpdir: tmpdir  # no artifact bucket here

F32 = mybir.dt.float32
F32R = mybir.dt.float32r
BF16 = mybir.dt.bfloat16
ACTF = mybir.ActivationFunctionType
ALU = mybir.AluOpType

B, L, C, H, D = 4, 4096, 1024, 16, 64
EPS = 1e-6
NCORES = 8
HPC = H // NCORES * 4          # 8 heads per core
F = HPC * D                    # 512 local feature cols
KC = C // 128                  # 8 contraction chunks
NBLK = L // 512                # 8 token blocks (phase A)
NST = L // 128                 # 32 token subtiles
FT = F // 128                  # 4 feature tiles (head pairs)

_PROG = None


def _emit_kv(nc, b, Ksbs, V1s, kvP0, kvP1):
    # one matmul per head PAIR: lhsT = K columns of both heads (128 wide),
    # rhs = [V1_even | V1_odd] (130 wide).  The diagonal blocks of the
    # (128, 130) output are kv_even / kv_odd (+ mask columns); off-diagonal
    # blocks are cross-head garbage that the kv_sb evacuation skips.
    for s in range(4):
        st = b * 4 + s
        for f in range(FT):
            kvp = kvP0 if f < 2 else kvP1
            nc.tensor.matmul(
                kvp[:, f % 2, :],
                Ksbs[s][:, ts(f, 128)],
                V1s[s][:, 2 * f:2 * f + 2, :].rearrange("p a b -> p (a b)"),
                start=(st == 0 and f % 2 == 0),
                stop=(st == NST - 1 and f % 2 == 1),
                skip_group_check=True)


def _build_program():
    nc = bacc.Bacc("TRN2", target_bir_lowering=False, debug=False,
                   num_devices=NCORES)

    xtq_d = nc.dram_tensor("xtq", [C, L], BF16, kind="ExternalInput")
    xtk_d = nc.dram_tensor("xtk", [C, L], BF16, kind="ExternalInput")
    xtv_d = nc.dram_tensor("xtv", [C, L], BF16, kind="ExternalInput")
    wtq_d = nc.dram_tensor("wtq", [C, F], BF16, kind="ExternalInput")
    wtk_d = nc.dram_tensor("wtk", [C, F], BF16, kind="ExternalInput")
    wtv_d = nc.dram_tensor("wtv", [C, F], BF16, kind="ExternalInput")
    bq_d = nc.dram_tensor("bq", [F], F32, kind="ExternalInput")
    bk_d = nc.dram_tensor("bk", [1, F], BF16, kind="ExternalInput")
    bv_d = nc.dram_tensor("bv", [1, F], BF16, kind="ExternalInput")
    mq_d = nc.dram_tensor("mq", [L], F32, kind="ExternalInput")
    mkv_d = nc.dram_tensor("mkv", [L], F32, kind="ExternalInput")
    out_d = nc.dram_tensor("out", [L, F], F32, kind="ExternalOutput")

    with tile.TileContext(nc) as tc:
        with (
            tc.tile_pool(name="singles", bufs=1) as singles,
            tc.tile_pool(name="xtp", bufs=6) as xtp,
            tc.tile_pool(name="tmp", bufs=4) as tmp,
            tc.tile_pool(name="kvout", bufs=4) as kvout,
        ):
            # constants / weights.  DMA order matters for the pipeline head:
            # the first matmuls need (wtk, xtk block 0) — load those first.
            xtq_ap = xtq_d.ap().rearrange("(o p) l -> p o l", p=128)
            xtk_ap = xtk_d.ap().rearrange("(o p) l -> p o l", p=128)
            xtv_ap = xtv_d.ap().rearrange("(o p) l -> p o l", p=128)

            bk_row = singles.tile([1, F], BF16)
            nc.sync.dma_start(bk_row[:], bk_d.ap())
            wtk_sb = singles.tile([128, KC, F], BF16)
            wtk_ap = wtk_d.ap().rearrange("(o p) f -> p o f", p=128)
            nc.sync.dma_start(wtk_sb[:, 0:2, :], wtk_ap[:, 0:2, :])
            xtk_t0 = xtp.tile([128, KC, 512], BF16, tag="xt")
            nc.sync.dma_start(xtk_t0[:, 0:2, :], xtk_ap[:, 0:2, ds(0, 512)])
            nc.sync.dma_start(wtk_sb[:, 2:4, :], wtk_ap[:, 2:4, :])
            nc.sync.dma_start(xtk_t0[:, 2:4, :], xtk_ap[:, 2:4, ds(0, 512)])
            nc.sync.dma_start(wtk_sb[:, 4:8, :], wtk_ap[:, 4:8, :])
            nc.sync.dma_start(xtk_t0[:, 4:8, :], xtk_ap[:, 4:8, ds(0, 512)])
            bv_row = singles.tile([1, F], BF16)
            nc.sync.dma_start(bv_row[:], bv_d.ap())
            wtv_sb = singles.tile([128, KC, F], BF16)
            nc.sync.dma_start(wtv_sb[:], wtv_d.ap().rearrange("(o p) f -> p o f", p=128))
            xtv_t0 = xtp.tile([128, KC, 512], BF16, tag="xt")
            nc.sync.dma_start(xtv_t0[:], xtv_ap[:, :, ds(0, 512)])
            bq_pp = singles.tile([128, FT], F32)
            nc.sync.dma_start(bq_pp[:], bq_d.ap().rearrange("(f p) -> p f", p=128))
            wtq_sb = singles.tile([128, KC, F], BF16)
            nc.sync.dma_start(wtq_sb[:], wtq_d.ap().rearrange("(o p) f -> p o f", p=128))
            xtq_t0 = xtp.tile([128, KC, 512], BF16, tag="xt")
            nc.sync.dma_start(xtq_t0[:], xtq_ap[:, :, ds(0, 512)])
            ones_row = singles.tile([1, 128], BF16)
            nc.vector.memset(ones_row[:], 1.0)
            mq_sb = singles.tile([128, NST], F32)
            nc.sync.dma_start(mq_sb[:], mq_d.ap().rearrange("(t p) -> p t", p=128))
            mkv_sb = singles.tile([128, NST], F32)
            nc.sync.dma_start(mkv_sb[:], mkv_d.ap().rearrange("(t p) -> p t", p=128))
            # derived mask tiles
            mm2L = singles.tile([128, NST], F32)   # m_kv^2 / L
            nc.vector.scalar_tensor_tensor(mm2L[:], mkv_sb[:], 1.0 / L, mkv_sb[:],
                                           ALU.mult, ALU.mult)
            mqL = singles.tile([128, NST], F32)    # m_q * L
            nc.vector.tensor_scalar(mqL[:], mq_sb[:], float(L), None, ALU.mult)
            mkv8 = singles.tile([128, NST, HPC], F32)  # mask col replicated per head
            for h in range(HPC):
                nc.vector.tensor_copy(out=mkv8[:, :, h:h + 1], in_=mkv_sb[:, :, None])

            QT = singles.tile([128, FT, L], BF16)  # resident Q^T (feat-major)

            with tc.tile_pool(name="psA", bufs=2, space="PSUM") as psA:
                kvP0 = psA.tile([128, 2, 130], F32, name="kvP0", bufs=1)
                kvP1 = psA.tile([128, 2, 130], F32, name="kvP1", bufs=1)
                for b in range(NBLK):
                    with nc.named_scope(f"blk{b}"):
                        if b == 0:
                            xtk_t, xtv_t, xtq_t = xtk_t0, xtv_t0, xtq_t0
                        else:
                            xtk_t = xtp.tile([128, KC, 512], BF16, tag="xt")
                            nc.sync.dma_start(xtk_t[:], xtk_ap[:, :, ds(b * 512, 512)])
                            xtv_t = xtp.tile([128, KC, 512], BF16, tag="xt")
                            nc.sync.dma_start(xtv_t[:], xtv_ap[:, :, ds(b * 512, 512)])
                            xtq_t = xtp.tile([128, KC, 512], BF16, tag="xt")
                            nc.sync.dma_start(xtq_t[:], xtq_ap[:, :, ds(b * 512, 512)])

                        # ---- K projections + feature map, 4 subtiles
                        Ksbs, V1s = [], []
                        for s in range(4):
                            psk = psA.tile([128, 512], F32, tag="psk")
                            for k in range(KC):
                                nc.tensor.matmul(psk[:], xtk_t[:, k, ts(s, 128)],
                                                 wtk_sb[:, k, :],
                                                 start=(k == 0), stop=False)
                            nc.tensor.matmul(psk[:], ones_row[:], bk_row[:],
                                             start=False, stop=True)
                            E2 = tmp.tile([128, 512], F32, tag="E")
                            nc.scalar.activation(E2[:], psk[:], ACTF.Exp)
                            R2 = tmp.tile([128, 512], F32, tag="R")
                            nc.scalar.activation(R2[:], psk[:], ACTF.Relu)
                            Ksb = tmp.tile([128, 512], BF16, tag="Ksb", bufs=6)
                            nc.vector.scalar_tensor_tensor(Ksb[:], E2[:], 1.0,
                                                           R2[:], ALU.min, ALU.add)
                            Ksbs.append(Ksb)

                        # ---- V projections + mask/scale, 4 subtiles
                        for s in range(4):
                            st = b * 4 + s
                            psv = psA.tile([128, 512], F32, tag="psv")
                            for k in range(KC):
                                nc.tensor.matmul(psv[:], xtv_t[:, k, ts(s, 128)],
                                                 wtv_sb[:, k, :],
                                                 start=(k == 0), stop=False)
                            nc.tensor.matmul(psv[:], ones_row[:], bv_row[:],
                                             start=False, stop=True)
                            V1 = tmp.tile([128, HPC, 65], BF16, tag="V1", bufs=6)
                            nc.vector.tensor_scalar(
                                V1[:, :, 0:64],
                                psv.rearrange("p (h d) -> p h d", d=64),
                                mm2L[:, st:st + 1], None, ALU.mult)
                            nc.vector.tensor_copy(out=V1[:, :, 64:65],
                                                  in_=mkv8[:, st, :, None])
                            V1s.append(V1)

                        # ---- kv for blocks 0 and 7 runs before Q (b0: Q data
                        # still in flight; b7: lets early phase-B iterations
                        # overlap the last Q projection)
                        if b in (0, NBLK - 1):
                            _emit_kv(nc, b, Ksbs, V1s, kvP0, kvP1)

                        # ---- Q^T tiles: (128 feat, 512 tok), weights stationary
                        if b == NBLK - 1:
                            last_q = (xtq_t, b)
                            continue
                        for f in range(FT):
                            psq = psA.tile([128, 512], F32, tag="psq")
                            for k in range(KC):
                                nc.tensor.matmul(psq[:], wtq_sb[:, k, ts(f, 128)],
                                                 xtq_t[:, k, :],
                                                 start=(k == 0), stop=(k == KC - 1))
                            E = tmp.tile([128, 512], F32, tag="E")
                            nc.scalar.activation(E[:], psq[:], ACTF.Exp,
                                                 bias=bq_pp[:, f:f + 1])
                            R = tmp.tile([128, 512], F32, tag="R")
                            nc.scalar.activation(R[:], psq[:], ACTF.Relu,
                                                 bias=bq_pp[:, f:f + 1])
                            nc.vector.scalar_tensor_tensor(
                                QT[:, f, ds(b * 512, 512)], E[:], 1.0, R[:],
                                ALU.min, ALU.add)

                        # ---- kv accumulation (blocks 1..6)
                        if b not in (0, NBLK - 1):
                            _emit_kv(nc, b, Ksbs, V1s, kvP0, kvP1)

                # block-diagonal kv for the output einsum:
                # rows 0:64 = even head of pair (cols 0:65), rows 64:128 = odd
                kv_sb = singles.tile([128, FT, 130], BF16)
                nc.vector.memset(kv_sb[:], 0.0)
                nc.vector.tensor_copy(out=kv_sb[0:64, 0:2, 0:65],
                                      in_=kvP0[0:64, :, 0:65])
                nc.vector.tensor_copy(out=kv_sb[64:128, 0:2, 65:130],
                                      in_=kvP0[64:128, :, 65:130])
                nc.vector.tensor_copy(out=kv_sb[0:64, 2:4, 0:65],
                                      in_=kvP1[0:64, :, 0:65])
                nc.vector.tensor_copy(out=kv_sb[64:128, 2:4, 65:130],
                                      in_=kvP1[64:128, :, 65:130])

            with tc.tile_pool(name="psB", bufs=2, space="PSUM") as psB, \
                 nc.named_scope("phaseB"):

                def emit_B(st):
                    poA = psB.tile([128, 2, 130], F32, tag="poA", bufs=4)
                    poB = psB.tile([128, 2, 130], F32, tag="poB", bufs=4)
                    for f in range(FT):
                        po = poA if f < 2 else poB
                        nc.tensor.matmul(po[:, f % 2, :],
                                         QT[:, f, ts(st, 128)],
                                         kv_sb[:, f, :],
                                         start=True, stop=True,
                                         skip_group_check=True)
                    qs8 = kvout.tile([128, HPC], F32, tag="qs8", bufs=4)
                    nc.vector.tensor_copy(
                        out=qs8[:, 0:4].rearrange("p (f c) -> p f c", c=2),
                        in_=poA[:, :, 64:130:65])
                    nc.vector.tensor_copy(
                        out=qs8[:, 4:8].rearrange("p (f c) -> p f c", c=2),
                        in_=poB[:, :, 64:130:65])
                    # s = m_q*L / (eps + m_q*qs), fused per-partition ops
                    den = kvout.tile([128, HPC], F32, tag="den", bufs=4)
                    nc.vector.tensor_scalar(den[:], qs8[:], mq_sb[:, st:st + 1],
                                            EPS, ALU.mult, ALU.add)
                    rcp = kvout.tile([128, HPC], F32, tag="rcp", bufs=4)
                    nc.vector.reciprocal(rcp[:], den[:])
                    s8 = kvout.tile([128, HPC], F32, tag="s8", bufs=4)
                    nc.vector.tensor_scalar(s8[:], rcp[:], mqL[:, st:st + 1],
                                            None, ALU.mult)
                    outsb = kvout.tile([128, HPC, 64], F32, tag="outsb", bufs=4)
                    # poA heads on DVE in one fused 4-D op; poB heads on ScalarE
                    nc.vector.tensor_tensor(
                        out=outsb[:, 0:4, :]
                            .rearrange("p (f par) d -> p f par d", par=2),
                        in0=poA.rearrange("p f (par c) -> p f par c", c=65)
                            [:, :, :, 0:64],
                        in1=s8[:, 0:4]
                            .rearrange("p (f par) -> p f par", par=2)
                            [:, :, :, None].to_broadcast((128, 2, 2, 64)),
                        op=ALU.mult)
                    nc.vector.tensor_tensor(
                        out=outsb[:, 4:6, :]
                            .rearrange("p (f par) d -> p f par d", par=2),
                        in0=poB[:, 0:1, :].rearrange("p f (par c) -> p f par c",
                                                     c=65)[:, :, :, 0:64],
                        in1=s8[:, 4:6]
                            .rearrange("p (f par) -> p f par", par=2)
                            [:, :, :, None].to_broadcast((128, 1, 2, 64)),
                        op=ALU.mult)
                    for h in range(6, 8):
                        par = h % 2
                        cols = slice(0, 64) if par == 0 else slice(65, 129)
                        nc.scalar.activation(outsb[:, h, :], poB[:, 1, cols],
                                             ACTF.Identity, bias=0.0,
                                             scale=s8[:, h:h + 1])
                    nc.sync.dma_start(out_d.ap()[ds(st * 128, 128), :],
                                      outsb.rearrange("p h d -> p (h d)"))

                # first few output tiles run before the deferred last-block Q
                # projection — their epilogues overlap its PE work
                for st in range(6):
                    emit_B(st)

                xtq_t7, b7 = last_q
                for f in range(FT):
                    psq = psB.tile([128, 512], F32, tag="poA", bufs=4)
                    for k in range(KC):
                        nc.tensor.matmul(psq[:], wtq_sb[:, k, ts(f, 128)],
                                         xtq_t7[:, k, :],
                                         start=(k == 0), stop=(k == KC - 1))
                    E = tmp.tile([128, 512], F32, tag="E")
                    nc.scalar.activation(E[:], psq[:], ACTF.Exp,
                                         bias=bq_pp[:, f:f + 1])
                    R = tmp.tile([128, 512], F32, tag="R")
                    nc.scalar.activation(R[:], psq[:], ACTF.Relu,
                                         bias=bq_pp[:, f:f + 1])
                    nc.vector.scalar_tensor_tensor(
                        QT[:, f, ds(b7 * 512, 512)], E[:], 1.0, R[:],
                        ALU.min, ALU.add)

                for st in range(6, NST):
                    emit_B(st)

    nc.compile()
    return nc


def _get_program():
    global _PROG
    if _PROG is None:
        _PROG = _build_program()
    return _PROG


def kernel(query, key, value, mask_q, mask_kv, Wq, bq, Wk, bk, Wv, bv):
    query = np.asarray(query, dtype=np.float32)
    key = np.asarray(key, dtype=np.float32)
    value = np.asarray(value, dtype=np.float32)
    mask_q = np.asarray(mask_q, dtype=np.float32)
    mask_kv = np.asarray(mask_kv, dtype=np.float32)
    Wq = np.asarray(Wq, dtype=np.float32)
    bq = np.asarray(bq, dtype=np.float32)
    Wk = np.asarray(Wk, dtype=np.float32)
    bk = np.asarray(bk, dtype=np.float32)
    Wv = np.asarray(Wv, dtype=np.float32)
    bv = np.asarray(bv, dtype=np.float32)

    nc = _get_program()

    bf = ml_dtypes.bfloat16
    xt = {}
    for b in range(B):
        xt[b] = (np.ascontiguousarray(query[b].T).astype(bf),
                 np.ascontiguousarray(key[b].T).astype(bf),
                 np.ascontiguousarray(value[b].T).astype(bf))
    wslices = {}
    for hh in range(2):
        sl = slice(hh * F, (hh + 1) * F)
        wslices[hh] = (
            np.ascontiguousarray(Wq[sl, :].T).astype(bf),
            np.ascontiguousarray(Wk[sl, :].T).astype(bf),
            np.ascontiguousarray(Wv[sl, :].T).astype(bf),
            bq[sl].copy(),
            bk[sl].astype(bf).reshape(1, F),
            bv[sl].astype(bf).reshape(1, F),
        )

    in_maps = []
    for core in range(NCORES):
        b, hh = core // 2, core % 2
        xtq, xtk, xtv = xt[b]
        wtq, wtk, wtv, bq_h, bk_h, bv_h = wslices[hh]
        in_maps.append({
            "xtq": xtq, "xtk": xtk, "xtv": xtv,
            "wtq": wtq, "wtk": wtk, "wtv": wtv,
            "bq": bq_h, "bk": bk_h, "bv": bv_h,
            "mq": mask_q[b], "mkv": mask_kv[b],
        })

    trace = os.environ.get("TRN_KERNEL_TRACE", "0") == "1"
    trace_cores = list(range(NCORES)) if trace else None
    res = run_bass_kernel_spmd(nc, in_maps, list(range(NCORES)),
                               trace=trace, trace_cores=trace_cores)
    if trace:
        kernel.last_exec_time_ns = res.exec_time_ns
        kernel.last_scope_times = res.per_core_scope_times

    out = np.empty((B, L, H, D), dtype=np.float32)
    for core in range(NCORES):
        b, hh = core // 2, core % 2
        out[b, :, hh * HPC:(hh + 1) * HPC, :] = \
            res.results[core]["out"].reshape(L, HPC, D)
    return out
